# revision 5
# baseline (speedup 1.0000x reference)
"""Trainium2 Bass kernel for nn_CrossAdjacencyMatrix (gnn_message_passing), v2.

Computes, for two independent sets (sr, tg):
    he, te, re = ent[h], ent[t], rel[r]                 (per-triple gathers)
    tv  = 1 - sum(|he + re - te|) * INV                 [N]
    A   = scatter(h,t){0.3*tv + 0.4*rel_w[r]}           [E,E] (positions unique)
    out = conf * imp * (0.3*pca + A) + I

Sharding: rows of the [E,E] adjacency split into 8 blocks of 625 rows (one
per NeuronCore); triples routed by head id on the host.  Embedding tables
replicated (DRAM, bf16).

Per core, per set, the triples are laid out m-major: slot (m, gr) is the
m-th triple of global row gr (0..639, padded to 5x128), so one dma_gather
call of 640 descriptors fetches column m's tail (or rel) embeddings into
the SWDGE-native layout [128, 5, D] with partition = row%128 and q = row
tile (two SWDGE queues: te on q0, re on q1, so the two transfers
pipeline).  Scores then reduce on the DVE along the free D axis with
|he5 + re - te| (he5 is a dense DMA of this core's own 640 ent rows,
reused across all m), writing [128, 5] per m straight into the per-row
v16 buffer -- no transposes, no PSUM, no departition step.

A is built as dense fp16 SBUF tiles by GPSIMD local_scatter (per-partition
tail indices, pads idx=-1), then the dense pass streams conf/imp/pca as
one packed fp16 [RB, 3, E] tensor (host-converted; whole 2500-col chunks,
10KB contiguous per partition) computing conf*imp*(0.3*pca + A) in fp16,
and the unit diagonal lands via per-tile 128-descriptor SWDGE indirect
scatter-adds (fp16 CCE add) after each tile's stores.

GPSIMD phases are ordered [gathers_a][scatters_a][gathers_b][scatters_b]
(3 library switches) so set a's dense pipeline overlaps set b's gathers.
"""

import numpy as np

E = 5000
D = 128
R = 1000
NCORES = 8
RB = E // NCORES          # 625 rows per core
NT = 5                    # row tiles per core: 4x128 + 113
NR = NT * 128             # 640 padded rows
CH = 1250                 # local_scatter num_elems (< 2048)
NCH = E // CH
DCH = 2500                # dense column chunk
NDCH = E // DCH
GI = 40                   # idx cols per 640-slot gather (640/16)
INV = 1.0 / (3.0 * float(np.sqrt(D)))

_CACHE = {}


def _wrap16(flat_idx):
    """SWDGE gather index layout: flat list -> [128, len/16] int16, idx j at
    [j%16, j//16], replicated across the eight 16-partition core groups."""
    n = len(flat_idx)
    assert n % 16 == 0
    t = np.zeros((128, n // 16), np.int16)
    t[:16] = np.asarray(flat_idx, np.int16).reshape(n // 16, 16).T
    for b in range(1, 8):
        t[b * 16 : (b + 1) * 16] = t[:16]
    return t


def _prep_set(ent, rel, rw, h, t, r, M, c):
    """Routed m-major triple data for one (core, set)."""
    import ml_dtypes

    h = np.asarray(h, np.int64)
    t = np.asarray(t, np.int64)
    r = np.asarray(r, np.int64)
    rw = np.asarray(rw, np.float32)
    sel = (h >= RB * c) & (h < RB * (c + 1))
    hl = (h[sel] - RB * c).astype(np.int64)
    tt = t[sel]
    rr = r[sel]
    # sort by (row, tail): within-row tail order makes each m-column's
    # gather read a narrow ascending band of the table (DRAM locality)
    order = np.lexsort((tt, hl))
    hl, tt, rr = hl[order], tt[order], rr[order]
    counts = np.bincount(hl, minlength=RB)
    starts = np.zeros(RB, np.int64)
    starts[1:] = np.cumsum(counts)[:-1]
    m_idx = np.arange(len(hl)) - starts[hl]
    assert counts.max() <= M, (counts.max(), M)

    tid = np.zeros((NR, M), np.int64)     # pad -> row 0 (garbage, killed by c0)
    rid = np.zeros((NR, M), np.int64)
    c0 = np.zeros((NR, M), np.float32)
    lsx = np.full((NR, M), -1, np.int64)  # local_scatter idx, pad -> -1
    tid[hl, m_idx] = tt
    rid[hl, m_idx] = rr
    c0[hl, m_idx] = 0.3 + 0.4 * rw[rr]
    lsx[hl, m_idx] = tt

    # gather indices: per m-column, te then re (rel rows at +E in tab)
    gix = np.zeros((128, M * 2 * GI), np.int16)
    for m in range(M):
        gix[:, 2 * GI * m : 2 * GI * m + GI] = _wrap16(tid[:, m])
        gix[:, 2 * GI * m + GI : 2 * GI * (m + 1)] = _wrap16(rid[:, m] + E)

    # v16 layout [128, NT, M]: row gr = 128q + p
    c0p = np.ascontiguousarray(
        c0.reshape(NT, 128, M).transpose(1, 0, 2).reshape(128, NT * M)
    ).astype(np.float16)

    # local_scatter chunk indices [128, NT*NCH*M]
    lsx5 = lsx.reshape(NT, 128, M)
    lidx = np.full((NT, NCH, 128, M), -1, np.int16)
    for k in range(NCH):
        rel_k = lsx5 - k * CH
        ink = (lsx5 >= k * CH) & (lsx5 < (k + 1) * CH)
        lidx[:, k][ink] = rel_k[ink].astype(np.int16)
    lidx = np.ascontiguousarray(
        lidx.transpose(2, 0, 1, 3).reshape(128, NT * NCH * M)
    )

    # tab rows: ent [0,E), rel [E,E+R), this core's he rows [E+R, E+R+NR)
    tab = np.zeros((E + R + NR, D), ml_dtypes.bfloat16)
    tab[:E] = ent.astype(ml_dtypes.bfloat16)
    tab[E : E + R] = rel.astype(ml_dtypes.bfloat16)
    tab[E + R : E + R + RB] = ent[RB * c : RB * (c + 1)].astype(ml_dtypes.bfloat16)

    return {"gix": gix, "c0": c0p, "lidx": lidx, "tab": tab}


def _patch_tile_tail():
    """This walrus build rejects instructions carrying more than one sync
    wait. Spread the Tile tail drain's sem waits across one nop each."""
    import concourse.tile as tile_mod
    import concourse.mybir as mybir
    from concourse.vector_clock import ScopedClock

    if getattr(tile_mod.TileContext, "_drain_patched", False):
        return

    def _patched(self, tick_clock, wait_clock):
        nc = self.nc
        nops = [nc.sync.nop(nofuse=True) for _ in range(8)]
        drain_inst = nc.sync.drain()
        wait_clock.add_sem_waits(
            drain_inst.ins, ScopedClock({None: tick_clock.global_clock})
        )
        waits = list(drain_inst.ins.sync_info.on_wait)
        if len(waits) > 1:
            drain_inst.ins.sync_info.on_wait = []
            for i, w in enumerate(waits):
                tgt = nops[i].ins if i < len(nops) else nc.sync.nop(nofuse=True).ins
                if tgt.sync_info is None:
                    tgt.sync_info = mybir.SyncInfo(on_wait=[], on_update=[])
                tgt.sync_info.on_wait = [w]
        nc.all_engine_barrier()
        assert self.sems is not None
        popped = nc._tile_sem_poison_stack.pop()
        assert popped is self._sem_poison
        nc.clear_and_free_semaphores(list(self.sems.allocated().values()))
        nc.all_engine_barrier()

    tile_mod.TileContext._drain_and_barrier = _patched
    tile_mod.TileContext._drain_patched = True


def _split_excess_waits(nc, limit=1):
    """Move excess sync waits onto same-engine InstNoOp instructions inserted
    immediately before the offender."""
    import concourse.mybir as mybir

    counter = [0]

    def fresh_nop(engine, wait):
        counter[0] += 1
        nop = mybir.InstNoOp(name=f"I-waitsplit-{counter[0]}", ins=[], outs=[])
        nop.engine = engine
        nop.sync_info = mybir.SyncInfo(on_wait=[wait], on_update=[])
        try:
            nc.register_instruction(nop, overwrite=True)
        except Exception:
            pass
        return nop

    for fn in nc.m.functions:
        for bb in fn.blocks:
            changed = False
            new_insts = []
            for inst in bb.instructions:
                si = getattr(inst, "sync_info", None)
                waits = list(si.on_wait) if si is not None and si.on_wait else []
                lim = 0 if inst.opcode == "Drain" else limit
                if len(waits) > lim:
                    excess = waits[: len(waits) - lim]
                    si.on_wait = waits[len(waits) - lim :]
                    for w in excess:
                        new_insts.append(fresh_nop(inst.engine, w))
                    changed = True
                new_insts.append(inst)
            if changed:
                bb.instructions = new_insts


def _finalize(nc):
    """Post-Tile passes: GPSIMD library loads, extended-ISA codegen, wait
    splitting."""
    import concourse.mybir as mybir
    from concourse.library_config import all_libraries, standard
    import bass_rust

    mask = {}
    for lib in all_libraries:
        for it in lib.instructions:
            mask[it] = mask.get(it, 0) | (1 << lib.index)
    bass_rust.insert_library_loads(nc, mask, len(all_libraries), standard.index)
    mybir.codegen_inst_isa_subclasses(nc)
    _split_excess_waits(nc)


def _build_nc(M):
    from concourse import bass, mybir
    import concourse.tile as tile
    from concourse.bass import IndirectOffsetOnAxis
    import bass_rust

    _patch_tile_tail()

    f32 = mybir.dt.float32
    f8 = mybir.dt.float8e4
    f16 = mybir.dt.float16
    bf16 = mybir.dt.bfloat16
    i32 = mybir.dt.int32
    i16 = mybir.dt.int16
    nc = bass.Bass(num_swdge_queues=4)
    T = {}
    for s in ("a", "b"):
        T[s] = dict(
            cip=nc.dram_tensor(f"cip_{s}", [RB, 3, E], f16, kind="ExternalInput"),
            tab=nc.dram_tensor(f"tab_{s}", [E + R + NR, D], bf16, kind="ExternalInput"),
            gix=nc.dram_tensor(f"gix_{s}", [128, M * 2 * GI], i16, kind="ExternalInput"),
            c0=nc.dram_tensor(f"c0_{s}", [128, NT * M], f16, kind="ExternalInput"),
            lidx=nc.dram_tensor(
                f"lidx_{s}", [128, NT * NCH * M], i16, kind="ExternalInput"
            ),
            out=nc.dram_tensor(f"out_{s}", [RB, E], f16, kind="ExternalOutput"),
        )
    d_dgi = nc.dram_tensor("dgi", [128, NT], i32, kind="ExternalInput")
    d_dgv = nc.dram_tensor("dgv", [128, NT], f16, kind="ExternalInput")

    gathers = {"a": [], "b": []}
    scatters = {"a": [], "b": []}
    _nireg = {}

    def nireg(n):
        if n not in _nireg:
            _nireg[n] = nc.gpsimd.to_reg(n)
        return _nireg[n]

    with tile.TileContext(nc) as tc:
        with (
            tc.tile_pool(name="fix", bufs=1) as pf,
            tc.tile_pool(name="gath", bufs=6) as pg,
            tc.tile_pool(name="vkeep", bufs=1) as pv,
            tc.tile_pool(name="amat", bufs=2) as pa,
            tc.tile_pool(name="dense", bufs=4) as pd,
            tc.tile_pool(name="outp", bufs=3) as po,
        ):
            dgit = pf.tile([128, NT], i32, tag="dgit")
            nc.sync.dma_start(out=dgit[:], in_=d_dgi[:])
            dgvt = pf.tile([128, NT], f16, tag="dgvt")
            nc.sync.dma_start(out=dgvt[:], in_=d_dgv[:])

            he5 = {}
            gixt = {}
            c0t = {}
            lidxt = {}
            v16 = {}
            v16f = {}
            for s in ("a", "b"):
                ts = T[s]
                he = pf.tile([128, NT, D], bf16, tag=f"he5_{s}", name=f"he5_{s}")
                nc.sync.dma_start(
                    out=he[:],
                    in_=ts["tab"][E + R : E + R + NR].rearrange(
                        "(q p) d -> p q d", p=128
                    ),
                )
                he5[s] = he
                gx = pf.tile([128, M * 2 * GI], i16, tag=f"gix_{s}")
                nc.sync.dma_start(out=gx[:], in_=ts["gix"][:])
                gixt[s] = gx
                c0 = pf.tile([128, NT * M], f16, tag=f"c0_{s}")
                nc.sync.dma_start(out=c0[:], in_=ts["c0"][:])
                c0t[s] = c0
                lx = pf.tile([128, NT * NCH * M], i16, tag=f"lidx_{s}")
                nc.sync.dma_start(out=lx[:], in_=ts["lidx"][:])
                lidxt[s] = lx
                v16[s] = pv.tile([128, NT, M], f32, tag=f"v16_{s}", name=f"v16_{s}")
                v16f[s] = pv.tile([128, NT * M], f16, tag=f"v16f_{s}", name=f"v16f_{s}")

            def gather_phase(s):
                ts = T[s]
                for m in range(M):
                    te = pg.tile([128, NT, D], bf16, tag="te")
                    re = pg.tile([128, NT, D], bf16, tag="re")
                    g1 = nc.gpsimd.dma_gather(
                        te[:], ts["tab"][:],
                        gixt[s][:, 2 * GI * m : 2 * GI * m + GI],
                        NR, nireg(NR), D, queue_num=(2 * (m % 2)),
                    )
                    g2 = nc.gpsimd.dma_gather(
                        re[:], ts["tab"][:],
                        gixt[s][:, 2 * GI * m + GI : 2 * GI * (m + 1)],
                        NR, nireg(NR), D, queue_num=(2 * (m % 2) + 1),
                    )
                    gathers[s] += [g1, g2]
                    nc.vector.tensor_tensor(
                        out=re[:], in0=re[:], in1=he5[s][:],
                        op=mybir.AluOpType.add,
                    )
                    nc.vector.tensor_tensor(
                        out=te[:], in0=re[:], in1=te[:],
                        op=mybir.AluOpType.subtract,
                    )
                    nc.vector.tensor_reduce(
                        out=v16[s][:, :, m],
                        in_=te[:],
                        axis=mybir.AxisListType.X,
                        op=mybir.AluOpType.add,
                        apply_absolute_value=True,
                    )
                # v = c0 - 0.3*INV*red
                nc.vector.scalar_tensor_tensor(
                    out=v16f[s][:],
                    in0=v16[s][:].rearrange("p q m -> p (q m)"),
                    scalar=-0.3 * INV,
                    in1=c0t[s][:],
                    op0=mybir.AluOpType.mult,
                    op1=mybir.AluOpType.add,
                )

            def scatter_dense_phase(s):
                ts = T[s]
                for ti in range(NT):
                    nrows = RB - 128 * ti if ti == NT - 1 else 128
                    rsl = slice(128 * ti, 128 * ti + nrows)
                    tile_writes = []
                    amat = pa.tile([128, E], f16, tag="amat")
                    ldeng = {0: nc.sync, 1: nc.scalar}
                    steng = {0: nc.scalar, 1: nc.sync}
                    for k in range(NCH):
                        sc = nc.gpsimd.local_scatter(
                            amat[:, CH * k : CH * (k + 1)],
                            v16f[s][:, M * ti : M * (ti + 1)],
                            lidxt[s][:, (NCH * ti + k) * M : (NCH * ti + k + 1) * M],
                            128,
                            CH,
                            M,
                        )
                        scatters[s].append(sc)
                    for k in range(NDCH):
                        csl = slice(DCH * k, DCH * (k + 1))
                        cip = pd.tile([128, 3, DCH], f16, tag="cip")
                        ldeng[k % 2].dma_start(
                            out=cip[:nrows], in_=ts["cip"][rsl, :, csl]
                        )
                        cf = cip[:, 0, :]
                        im = cip[:, 1, :]
                        pc = cip[:, 2, :]
                        pc2 = po.tile([128, DCH], f16, tag="pc2")
                        nc.vector.scalar_tensor_tensor(
                            out=pc2[:nrows], in0=pc[:nrows], scalar=0.3,
                            in1=amat[:nrows, csl],
                            op0=mybir.AluOpType.mult, op1=mybir.AluOpType.add,
                        )
                        nc.vector.tensor_tensor(
                            out=cf[:nrows], in0=cf[:nrows], in1=im[:nrows],
                            op=mybir.AluOpType.mult,
                        )
                        ot = po.tile([128, DCH], f16, tag="ot")
                        nc.vector.tensor_tensor(
                            out=ot[:nrows], in0=cf[:nrows], in1=pc2[:nrows],
                            op=mybir.AluOpType.mult,
                        )
                        w = steng[k % 2].dma_start(
                            out=ts["out"][rsl, csl], in_=ot[:nrows]
                        )
                        tile_writes.append(w)
                    diag_work.append((s, ti, tile_writes))

            diag_work = []
            gather_phase("a")
            scatter_dense_phase("a")
            gather_phase("b")
            scatter_dense_phase("b")
            # unit diagonal: per-tile single-offset indirect scatter-adds
            # (multi-column offset APs scatter contiguously -- wrong), emitted
            # LAST so the in-order Pool queue never stalls set b's gathers
            # behind set a's dense stores
            diags = []
            for s, ti, tile_writes in diag_work:
                ts = T[s]
                out_flat = ts["out"][:].rearrange("a b -> (a b)").unsqueeze(1)
                dsc = nc.gpsimd.indirect_dma_start(
                    out=out_flat,
                    out_offset=IndirectOffsetOnAxis(ap=dgit[:, ti : ti + 1], axis=0),
                    in_=dgvt[:, ti : ti + 1],
                    in_offset=None,
                    compute_op=mybir.AluOpType.add,
                )
                diags.append(dsc)
                for w in tile_writes:
                    bass_rust.add_dep_helper(dsc.ins, w.ins, True, "diag after dense")

        # Pool ordering: scatters_a after gathers_a is natural (v16f); force
        # gathers_b after scatters_a so set a's dense work overlaps set b's
        # gathers with exactly 3 library switches; scatters_b after gathers_b
        # is natural again.
        for g in gathers["b"]:
            for sc in scatters["a"]:
                bass_rust.add_dep_helper(g.ins, sc.ins, False, "lib phase order")
        for d in diags:
            for g in gathers["b"]:
                bass_rust.add_dep_helper(d.ins, g.ins, False, "diag last")
            for sc in scatters["b"]:
                bass_rust.add_dep_helper(d.ins, sc.ins, False, "diag last")

    _finalize(nc)
    return nc


class _Runner:
    """Compile the SPMD bass program through PJRT once; keep the jitted
    callable for repeated execution."""

    def __init__(self, nc):
        import jax
        from jax.sharding import Mesh, PartitionSpec
        from jax.experimental.shard_map import shard_map
        from concourse import bass2jax
        import concourse.mybir as _mybir

        bass2jax.install_neuronx_cc_hook()
        self.jax = jax
        partition_name = (
            nc.partition_id_tensor.name if nc.partition_id_tensor else None
        )
        in_names, out_names, out_avals = [], [], []
        for alloc in nc.m.functions[0].allocations:
            if not isinstance(alloc, _mybir.MemoryLocationSet):
                continue
            name = alloc.memorylocations[0].name
            if alloc.kind == "ExternalInput":
                if name != partition_name:
                    in_names.append(name)
            elif alloc.kind == "ExternalOutput":
                out_names.append(name)
                out_avals.append(
                    jax.core.ShapedArray(
                        tuple(alloc.tensor_shape), _mybir.dt.np(alloc.dtype)
                    )
                )
        self.in_names, self.out_names, self.out_avals = in_names, out_names, out_avals

        bind_in_names = tuple(in_names) + (
            (partition_name,) if partition_name else ()
        )

        def _body(*args):
            operands = list(args)
            if partition_name is not None:
                operands.append(bass2jax.partition_id_tensor())
            outs = bass2jax._bass_exec_p.bind(
                *operands,
                out_avals=tuple(out_avals),
                in_names=bind_in_names,
                out_names=tuple(out_names),
                lowering_input_output_aliases=(),
                sim_require_finite=True,
                sim_require_nnan=True,
                nc=nc,
            )
            return tuple(outs)

        devices = jax.devices()[:NCORES]
        self.mesh = Mesh(np.asarray(devices), ("core",))
        in_specs = (PartitionSpec("core"),) * len(in_names)
        out_specs = (PartitionSpec("core"),) * len(out_names)
        self.fn = jax.jit(
            shard_map(
                _body,
                mesh=self.mesh,
                in_specs=in_specs,
                out_specs=out_specs,
                check_rep=False,
            ),
            keep_unused=True,
        )

    def concat_inputs(self, in_maps):
        return [
            np.concatenate([np.asarray(in_maps[c][n]) for c in range(NCORES)], axis=0)
            for n in self.in_names
        ]

    def run(self, concat_in):
        return self.fn(*concat_in)

    def split_outputs(self, out_arrs):
        res = []
        for c in range(NCORES):
            res.append(
                {
                    n: np.asarray(out_arrs[i]).reshape(
                        NCORES, *self.out_avals[i].shape
                    )[c]
                    for i, n in enumerate(self.out_names)
                }
            )
        return res


def _get_runner(M):
    key = ("runner", M)
    if key not in _CACHE:
        nc = _build_nc(M)
        _CACHE[key] = _Runner(nc)
    return _CACHE[key]


def _host_prep(inputs):
    ent = {
        "a": np.asarray(inputs["ent_emb_sr"], np.float32),
        "b": np.asarray(inputs["ent_emb_tg"], np.float32),
    }
    rel = {
        "a": np.asarray(inputs["rel_emb_sr"], np.float32),
        "b": np.asarray(inputs["rel_emb_tg"], np.float32),
    }

    Ms = []
    for sfx in ("sr", "tg"):
        h = np.asarray(inputs[f"head_{sfx}"], np.int64)
        Ms.append(
            max(
                np.bincount(
                    h[(h >= RB * c) & (h < RB * (c + 1))] - RB * c, minlength=RB
                ).max()
                for c in range(NCORES)
            )
        )
    M = int(max(Ms))
    M += M & 1

    # per-core diag offsets (element index into [RB*E] fp16 out); pads add
    # 0.0 at element 1 (diag element index r*E + RB*c + r == 1 requires
    # RB*c == 1, never true)
    dgi_all, dgv_all = [], []
    for c in range(NCORES):
        dgi = np.full((128, NT), 1, np.int32)
        dgv = np.zeros((128, NT), np.float16)
        for ti in range(NT):
            rows = np.arange(ti * 128, min((ti + 1) * 128, RB), dtype=np.int64)
            p = rows - ti * 128
            dgi[p, ti] = (rows * E + RB * c + rows).astype(np.int32)
            dgv[p, ti] = 1.0
        dgi_all.append(dgi)
        dgv_all.append(dgv)

    in_maps = [dict() for _ in range(NCORES)]
    for s in ("a", "b"):
        sfx = "sr" if s == "a" else "tg"
        conf = np.asarray(inputs[f"conf_{sfx}"], np.float32)
        imp = np.asarray(inputs[f"imp_{sfx}"], np.float32)
        pca = np.asarray(inputs[f"pca_{sfx}"], np.float32)
        cip = np.empty((E, 3, E), np.float16)
        cip[:, 0, :] = conf
        cip[:, 1, :] = imp
        cip[:, 2, :] = pca
        for c in range(NCORES):
            m = in_maps[c]
            prep = _prep_set(
                ent[s],
                rel[s],
                inputs[f"relation_w_{sfx}"],
                inputs[f"head_{sfx}"],
                inputs[f"tail_{sfx}"],
                inputs[f"rel_{sfx}"],
                M,
                c,
            )
            m[f"cip_{s}"] = cip[RB * c : RB * (c + 1)]
            m[f"tab_{s}"] = prep["tab"]
            m[f"gix_{s}"] = prep["gix"]
            m[f"c0_{s}"] = prep["c0"]
            m[f"lidx_{s}"] = prep["lidx"]
            m["dgi"] = dgi_all[c]
            m["dgv"] = dgv_all[c]
    return M, in_maps


def kernel(**inputs):
    M, in_maps = _host_prep(inputs)
    runner = _get_runner(M)
    concat_in = runner.concat_inputs(in_maps)
    out_arrs = runner.run(concat_in)
    res = runner.split_outputs(out_arrs)
    adj_sr = np.concatenate(
        [res[c]["out_a"].astype(np.float32) for c in range(NCORES)], axis=0
    )
    adj_tg = np.concatenate(
        [res[c]["out_b"].astype(np.float32) for c in range(NCORES)], axis=0
    )
    return adj_sr, adj_tg


# revision 6
# speedup vs baseline: 1.0445x; 1.0445x over previous
"""Trainium2 Bass kernel for nn_CrossAdjacencyMatrix (gnn_message_passing), v2.

Computes, for two independent sets (sr, tg):
    he, te, re = ent[h], ent[t], rel[r]                 (per-triple gathers)
    tv  = 1 - sum(|he + re - te|) * INV                 [N]
    A   = scatter(h,t){0.3*tv + 0.4*rel_w[r]}           [E,E] (positions unique)
    out = conf * imp * (0.3*pca + A) + I

Sharding: rows of the [E,E] adjacency split into 8 blocks of 625 rows (one
per NeuronCore); triples routed by head id on the host.  Embedding tables
replicated (DRAM, bf16).

Per core, per set, the triples are laid out m-major: slot (m, gr) is the
m-th triple of global row gr (0..639, padded to 5x128), so one dma_gather
call of 640 descriptors fetches column m's tail (or rel) embeddings into
the SWDGE-native layout [128, 5, D] with partition = row%128 and q = row
tile (two SWDGE queues: te on q0, re on q1, so the two transfers
pipeline).  Scores then reduce on the DVE along the free D axis with
|he5 + re - te| (he5 is a dense DMA of this core's own 640 ent rows,
reused across all m), writing [128, 5] per m straight into the per-row
v16 buffer -- no transposes, no PSUM, no departition step.

A is built as dense fp16 SBUF tiles by GPSIMD local_scatter (per-partition
tail indices, pads idx=-1), then the dense pass streams conf/imp/pca as
one packed fp16 [RB, 3, E] tensor (host-converted; whole 2500-col chunks,
10KB contiguous per partition) computing conf*imp*(0.3*pca + A) in fp16,
and the unit diagonal lands via per-tile 128-descriptor SWDGE indirect
scatter-adds (fp16 CCE add) after each tile's stores.

GPSIMD phases are ordered [gathers_a][scatters_a][gathers_b][scatters_b]
(3 library switches) so set a's dense pipeline overlaps set b's gathers.
"""

import numpy as np

E = 5000
D = 128
R = 1000
NCORES = 8
RB = E // NCORES          # 625 rows per core
NT = 5                    # row tiles per core: 4x128 + 113
NR = NT * 128             # 640 padded rows
# local_scatter chunk starts/widths (num_elems < 2048, even)
CHS = ((0, 2046), (2046, 2046), (4092, 908))
NCH = len(CHS)
DCH = 2500                # dense column chunk
NDCH = E // DCH
GI = 40                   # idx cols per 640-slot gather (640/16)
INV = 1.0 / (3.0 * float(np.sqrt(D)))

_CACHE = {}


def _wrap16(flat_idx):
    """SWDGE gather index layout: flat list -> [128, len/16] int16, idx j at
    [j%16, j//16], replicated across the eight 16-partition core groups."""
    n = len(flat_idx)
    assert n % 16 == 0
    t = np.zeros((128, n // 16), np.int16)
    t[:16] = np.asarray(flat_idx, np.int16).reshape(n // 16, 16).T
    for b in range(1, 8):
        t[b * 16 : (b + 1) * 16] = t[:16]
    return t


def _prep_set(ent, rel, rw, h, t, r, M, c):
    """Routed m-major triple data for one (core, set)."""
    import ml_dtypes

    h = np.asarray(h, np.int64)
    t = np.asarray(t, np.int64)
    r = np.asarray(r, np.int64)
    rw = np.asarray(rw, np.float32)
    sel = (h >= RB * c) & (h < RB * (c + 1))
    hl = (h[sel] - RB * c).astype(np.int64)
    tt = t[sel]
    rr = r[sel]
    # sort by (row, tail): within-row tail order makes each m-column's
    # gather read a narrow ascending band of the table (DRAM locality)
    order = np.lexsort((tt, hl))
    hl, tt, rr = hl[order], tt[order], rr[order]
    counts = np.bincount(hl, minlength=RB)
    starts = np.zeros(RB, np.int64)
    starts[1:] = np.cumsum(counts)[:-1]
    m_idx = np.arange(len(hl)) - starts[hl]
    assert counts.max() <= M, (counts.max(), M)

    tid = np.zeros((NR, M), np.int64)     # pad -> row 0 (garbage, killed by c0)
    rid = np.zeros((NR, M), np.int64)
    c0 = np.zeros((NR, M), np.float32)
    lsx = np.full((NR, M), -1, np.int64)  # local_scatter idx, pad -> -1
    tid[hl, m_idx] = tt
    rid[hl, m_idx] = rr
    c0[hl, m_idx] = 0.3 + 0.4 * rw[rr]
    lsx[hl, m_idx] = tt

    # gather indices: per m-column, te then re (rel rows at +E in tab)
    gix = np.zeros((128, M * 2 * GI), np.int16)
    for m in range(M):
        gix[:, 2 * GI * m : 2 * GI * m + GI] = _wrap16(tid[:, m])
        gix[:, 2 * GI * m + GI : 2 * GI * (m + 1)] = _wrap16(rid[:, m] + E)

    # v16 layout [128, NT, M]: row gr = 128q + p
    c0p = np.ascontiguousarray(
        c0.reshape(NT, 128, M).transpose(1, 0, 2).reshape(128, NT * M)
    ).astype(np.float16)

    # local_scatter chunk indices [128, NT*NCH*M]
    lsx5 = lsx.reshape(NT, 128, M)
    lidx = np.full((NT, NCH, 128, M), -1, np.int16)
    for k, (c0_, w_) in enumerate(CHS):
        rel_k = lsx5 - c0_
        ink = (lsx5 >= c0_) & (lsx5 < c0_ + w_)
        lidx[:, k][ink] = rel_k[ink].astype(np.int16)
    lidx = np.ascontiguousarray(
        lidx.transpose(2, 0, 1, 3).reshape(128, NT * NCH * M)
    )

    # tab rows: ent [0,E), rel [E,E+R), this core's he rows [E+R, E+R+NR)
    tab = np.zeros((E + R + NR, D), ml_dtypes.bfloat16)
    tab[:E] = ent.astype(ml_dtypes.bfloat16)
    tab[E : E + R] = rel.astype(ml_dtypes.bfloat16)
    tab[E + R : E + R + RB] = ent[RB * c : RB * (c + 1)].astype(ml_dtypes.bfloat16)

    return {"gix": gix, "c0": c0p, "lidx": lidx, "tab": tab}


def _patch_tile_tail():
    """This walrus build rejects instructions carrying more than one sync
    wait. Spread the Tile tail drain's sem waits across one nop each."""
    import concourse.tile as tile_mod
    import concourse.mybir as mybir
    from concourse.vector_clock import ScopedClock

    if getattr(tile_mod.TileContext, "_drain_patched", False):
        return

    def _patched(self, tick_clock, wait_clock):
        nc = self.nc
        nops = [nc.sync.nop(nofuse=True) for _ in range(8)]
        drain_inst = nc.sync.drain()
        wait_clock.add_sem_waits(
            drain_inst.ins, ScopedClock({None: tick_clock.global_clock})
        )
        waits = list(drain_inst.ins.sync_info.on_wait)
        if len(waits) > 1:
            drain_inst.ins.sync_info.on_wait = []
            for i, w in enumerate(waits):
                tgt = nops[i].ins if i < len(nops) else nc.sync.nop(nofuse=True).ins
                if tgt.sync_info is None:
                    tgt.sync_info = mybir.SyncInfo(on_wait=[], on_update=[])
                tgt.sync_info.on_wait = [w]
        nc.all_engine_barrier()
        assert self.sems is not None
        popped = nc._tile_sem_poison_stack.pop()
        assert popped is self._sem_poison
        nc.clear_and_free_semaphores(list(self.sems.allocated().values()))
        nc.all_engine_barrier()

    tile_mod.TileContext._drain_and_barrier = _patched
    tile_mod.TileContext._drain_patched = True


def _split_excess_waits(nc, limit=1):
    """Move excess sync waits onto same-engine InstNoOp instructions inserted
    immediately before the offender."""
    import concourse.mybir as mybir

    counter = [0]

    def fresh_nop(engine, wait):
        counter[0] += 1
        nop = mybir.InstNoOp(name=f"I-waitsplit-{counter[0]}", ins=[], outs=[])
        nop.engine = engine
        nop.sync_info = mybir.SyncInfo(on_wait=[wait], on_update=[])
        try:
            nc.register_instruction(nop, overwrite=True)
        except Exception:
            pass
        return nop

    for fn in nc.m.functions:
        for bb in fn.blocks:
            changed = False
            new_insts = []
            for inst in bb.instructions:
                si = getattr(inst, "sync_info", None)
                waits = list(si.on_wait) if si is not None and si.on_wait else []
                lim = 0 if inst.opcode == "Drain" else limit
                if len(waits) > lim:
                    excess = waits[: len(waits) - lim]
                    si.on_wait = waits[len(waits) - lim :]
                    for w in excess:
                        new_insts.append(fresh_nop(inst.engine, w))
                    changed = True
                new_insts.append(inst)
            if changed:
                bb.instructions = new_insts


def _finalize(nc):
    """Post-Tile passes: GPSIMD library loads, extended-ISA codegen, wait
    splitting."""
    import concourse.mybir as mybir
    from concourse.library_config import all_libraries, standard
    import bass_rust

    mask = {}
    for lib in all_libraries:
        for it in lib.instructions:
            mask[it] = mask.get(it, 0) | (1 << lib.index)
    bass_rust.insert_library_loads(nc, mask, len(all_libraries), standard.index)
    mybir.codegen_inst_isa_subclasses(nc)
    _split_excess_waits(nc)


def _build_nc(M):
    from concourse import bass, mybir
    import concourse.tile as tile
    from concourse.bass import IndirectOffsetOnAxis
    import bass_rust

    _patch_tile_tail()

    f32 = mybir.dt.float32
    f8 = mybir.dt.float8e4
    f16 = mybir.dt.float16
    bf16 = mybir.dt.bfloat16
    i32 = mybir.dt.int32
    i16 = mybir.dt.int16
    nc = bass.Bass(num_swdge_queues=4)
    T = {}
    for s in ("a", "b"):
        T[s] = dict(
            cip=nc.dram_tensor(f"cip_{s}", [RB, 3, E], f16, kind="ExternalInput"),
            tab=nc.dram_tensor(f"tab_{s}", [E + R + NR, D], bf16, kind="ExternalInput"),
            gix=nc.dram_tensor(f"gix_{s}", [128, M * 2 * GI], i16, kind="ExternalInput"),
            c0=nc.dram_tensor(f"c0_{s}", [128, NT * M], f16, kind="ExternalInput"),
            lidx=nc.dram_tensor(
                f"lidx_{s}", [128, NT * NCH * M], i16, kind="ExternalInput"
            ),
            out=nc.dram_tensor(f"out_{s}", [RB, E], f16, kind="ExternalOutput"),
        )
    d_dgi = nc.dram_tensor("dgi", [128, NT], i32, kind="ExternalInput")
    d_dgv = nc.dram_tensor("dgv", [128, NT], f16, kind="ExternalInput")

    gathers = {"a": [], "b": []}
    scatters = {"a": [], "b": []}
    _nireg = {}

    def nireg(n):
        if n not in _nireg:
            _nireg[n] = nc.gpsimd.to_reg(n)
        return _nireg[n]

    with tile.TileContext(nc) as tc:
        with (
            tc.tile_pool(name="fix", bufs=1) as pf,
            tc.tile_pool(name="gath", bufs=6) as pg,
            tc.tile_pool(name="vkeep", bufs=1) as pv,
            tc.tile_pool(name="amat", bufs=2) as pa,
            tc.tile_pool(name="dense", bufs=6) as pd,
        ):
            dgit = pf.tile([128, NT], i32, tag="dgit")
            nc.sync.dma_start(out=dgit[:], in_=d_dgi[:])
            dgvt = pf.tile([128, NT], f16, tag="dgvt")
            nc.sync.dma_start(out=dgvt[:], in_=d_dgv[:])

            he5 = {}
            gixt = {}
            c0t = {}
            lidxt = {}
            v16 = {}
            v16f = {}
            for s in ("a", "b"):
                ts = T[s]
                he = pf.tile([128, NT, D], bf16, tag=f"he5_{s}", name=f"he5_{s}")
                nc.sync.dma_start(
                    out=he[:],
                    in_=ts["tab"][E + R : E + R + NR].rearrange(
                        "(q p) d -> p q d", p=128
                    ),
                )
                he5[s] = he
                gx = pf.tile([128, M * 2 * GI], i16, tag=f"gix_{s}")
                nc.sync.dma_start(out=gx[:], in_=ts["gix"][:])
                gixt[s] = gx
                c0 = pf.tile([128, NT * M], f16, tag=f"c0_{s}")
                nc.sync.dma_start(out=c0[:], in_=ts["c0"][:])
                c0t[s] = c0
                lx = pf.tile([128, NT * NCH * M], i16, tag=f"lidx_{s}")
                nc.sync.dma_start(out=lx[:], in_=ts["lidx"][:])
                lidxt[s] = lx
                v16[s] = pv.tile([128, NT, M], f32, tag=f"v16_{s}", name=f"v16_{s}")
                v16f[s] = pv.tile([128, NT * M], f16, tag=f"v16f_{s}", name=f"v16f_{s}")

            def gather_phase(s):
                ts = T[s]
                for m in range(M):
                    te = pg.tile([128, NT, D], bf16, tag="te")
                    re = pg.tile([128, NT, D], bf16, tag="re")
                    g1 = nc.gpsimd.dma_gather(
                        te[:], ts["tab"][:],
                        gixt[s][:, 2 * GI * m : 2 * GI * m + GI],
                        NR, nireg(NR), D, queue_num=(2 * (m % 2)),
                    )
                    g2 = nc.gpsimd.dma_gather(
                        re[:], ts["tab"][:],
                        gixt[s][:, 2 * GI * m + GI : 2 * GI * (m + 1)],
                        NR, nireg(NR), D, queue_num=(2 * (m % 2) + 1),
                    )
                    gathers[s] += [g1, g2]
                    nc.vector.tensor_tensor(
                        out=re[:], in0=re[:], in1=he5[s][:],
                        op=mybir.AluOpType.add,
                    )
                    nc.vector.tensor_tensor(
                        out=te[:], in0=re[:], in1=te[:],
                        op=mybir.AluOpType.subtract,
                    )
                    nc.vector.tensor_reduce(
                        out=v16[s][:, :, m],
                        in_=te[:],
                        axis=mybir.AxisListType.X,
                        op=mybir.AluOpType.add,
                        apply_absolute_value=True,
                    )
                # v = c0 - 0.3*INV*red
                nc.vector.scalar_tensor_tensor(
                    out=v16f[s][:],
                    in0=v16[s][:].rearrange("p q m -> p (q m)"),
                    scalar=-0.3 * INV,
                    in1=c0t[s][:],
                    op0=mybir.AluOpType.mult,
                    op1=mybir.AluOpType.add,
                )

            def scatter_dense_phase(s):
                ts = T[s]
                for ti in range(NT):
                    nrows = RB - 128 * ti if ti == NT - 1 else 128
                    rsl = slice(128 * ti, 128 * ti + nrows)
                    tile_writes = []
                    amat = pa.tile([128, E], f16, tag="amat")
                    ldeng = {0: nc.sync, 1: nc.scalar}
                    steng = {0: nc.scalar, 1: nc.sync}
                    for k, (c0_, w_) in enumerate(CHS):
                        sc = nc.gpsimd.local_scatter(
                            amat[:, c0_ : c0_ + w_],
                            v16f[s][:, M * ti : M * (ti + 1)],
                            lidxt[s][:, (NCH * ti + k) * M : (NCH * ti + k + 1) * M],
                            128,
                            w_,
                            M,
                        )
                        scatters[s].append(sc)
                    for k in range(NDCH):
                        csl = slice(DCH * k, DCH * (k + 1))
                        cip = pd.tile([128, 3, DCH], f16, tag="cip")
                        ldeng[k % 2].dma_start(
                            out=cip[:nrows], in_=ts["cip"][rsl, :, csl]
                        )
                        cf = cip[:, 0, :]
                        im = cip[:, 1, :]
                        pc = cip[:, 2, :]
                        # compute in place in the (dead-after-use) amat slice:
                        # amat <- 0.3*pca + A, cf <- cf*im, amat <- cf*amat
                        nc.vector.scalar_tensor_tensor(
                            out=amat[:nrows, csl], in0=pc[:nrows], scalar=0.3,
                            in1=amat[:nrows, csl],
                            op0=mybir.AluOpType.mult, op1=mybir.AluOpType.add,
                        )
                        nc.vector.tensor_tensor(
                            out=cf[:nrows], in0=cf[:nrows], in1=im[:nrows],
                            op=mybir.AluOpType.mult,
                        )
                        nc.vector.tensor_tensor(
                            out=amat[:nrows, csl], in0=cf[:nrows],
                            in1=amat[:nrows, csl],
                            op=mybir.AluOpType.mult,
                        )
                        w = steng[k % 2].dma_start(
                            out=ts["out"][rsl, csl], in_=amat[:nrows, csl]
                        )
                        tile_writes.append(w)
                    diag_work.append((s, ti, tile_writes))

            diag_work = []
            gather_phase("a")
            scatter_dense_phase("a")
            gather_phase("b")
            scatter_dense_phase("b")
            # unit diagonal: per-tile single-offset indirect scatter-adds
            # (multi-column offset APs scatter contiguously -- wrong), emitted
            # LAST so the in-order Pool queue never stalls set b's gathers
            # behind set a's dense stores
            diags = []
            for s, ti, tile_writes in diag_work:
                ts = T[s]
                out_flat = ts["out"][:].rearrange("a b -> (a b)").unsqueeze(1)
                dsc = nc.gpsimd.indirect_dma_start(
                    out=out_flat,
                    out_offset=IndirectOffsetOnAxis(ap=dgit[:, ti : ti + 1], axis=0),
                    in_=dgvt[:, ti : ti + 1],
                    in_offset=None,
                    compute_op=mybir.AluOpType.add,
                )
                diags.append(dsc)
                for w in tile_writes:
                    bass_rust.add_dep_helper(dsc.ins, w.ins, True, "diag after dense")

        # Pool ordering: scatters_a after gathers_a is natural (v16f); force
        # gathers_b after scatters_a so set a's dense work overlaps set b's
        # gathers with exactly 3 library switches; scatters_b after gathers_b
        # is natural again.
        for g in gathers["b"]:
            for sc in scatters["a"]:
                bass_rust.add_dep_helper(g.ins, sc.ins, False, "lib phase order")
        for d in diags:
            for g in gathers["b"]:
                bass_rust.add_dep_helper(d.ins, g.ins, False, "diag last")
            for sc in scatters["b"]:
                bass_rust.add_dep_helper(d.ins, sc.ins, False, "diag last")

    _finalize(nc)
    return nc


class _Runner:
    """Compile the SPMD bass program through PJRT once; keep the jitted
    callable for repeated execution."""

    def __init__(self, nc):
        import jax
        from jax.sharding import Mesh, PartitionSpec
        from jax.experimental.shard_map import shard_map
        from concourse import bass2jax
        import concourse.mybir as _mybir

        bass2jax.install_neuronx_cc_hook()
        self.jax = jax
        partition_name = (
            nc.partition_id_tensor.name if nc.partition_id_tensor else None
        )
        in_names, out_names, out_avals = [], [], []
        for alloc in nc.m.functions[0].allocations:
            if not isinstance(alloc, _mybir.MemoryLocationSet):
                continue
            name = alloc.memorylocations[0].name
            if alloc.kind == "ExternalInput":
                if name != partition_name:
                    in_names.append(name)
            elif alloc.kind == "ExternalOutput":
                out_names.append(name)
                out_avals.append(
                    jax.core.ShapedArray(
                        tuple(alloc.tensor_shape), _mybir.dt.np(alloc.dtype)
                    )
                )
        self.in_names, self.out_names, self.out_avals = in_names, out_names, out_avals

        bind_in_names = tuple(in_names) + (
            (partition_name,) if partition_name else ()
        )

        def _body(*args):
            operands = list(args)
            if partition_name is not None:
                operands.append(bass2jax.partition_id_tensor())
            outs = bass2jax._bass_exec_p.bind(
                *operands,
                out_avals=tuple(out_avals),
                in_names=bind_in_names,
                out_names=tuple(out_names),
                lowering_input_output_aliases=(),
                sim_require_finite=True,
                sim_require_nnan=True,
                nc=nc,
            )
            return tuple(outs)

        devices = jax.devices()[:NCORES]
        self.mesh = Mesh(np.asarray(devices), ("core",))
        in_specs = (PartitionSpec("core"),) * len(in_names)
        out_specs = (PartitionSpec("core"),) * len(out_names)
        self.fn = jax.jit(
            shard_map(
                _body,
                mesh=self.mesh,
                in_specs=in_specs,
                out_specs=out_specs,
                check_rep=False,
            ),
            keep_unused=True,
        )

    def concat_inputs(self, in_maps):
        return [
            np.concatenate([np.asarray(in_maps[c][n]) for c in range(NCORES)], axis=0)
            for n in self.in_names
        ]

    def run(self, concat_in):
        return self.fn(*concat_in)

    def split_outputs(self, out_arrs):
        res = []
        for c in range(NCORES):
            res.append(
                {
                    n: np.asarray(out_arrs[i]).reshape(
                        NCORES, *self.out_avals[i].shape
                    )[c]
                    for i, n in enumerate(self.out_names)
                }
            )
        return res


def _get_runner(M):
    key = ("runner", M)
    if key not in _CACHE:
        nc = _build_nc(M)
        _CACHE[key] = _Runner(nc)
    return _CACHE[key]


def _host_prep(inputs):
    ent = {
        "a": np.asarray(inputs["ent_emb_sr"], np.float32),
        "b": np.asarray(inputs["ent_emb_tg"], np.float32),
    }
    rel = {
        "a": np.asarray(inputs["rel_emb_sr"], np.float32),
        "b": np.asarray(inputs["rel_emb_tg"], np.float32),
    }

    Ms = []
    for sfx in ("sr", "tg"):
        h = np.asarray(inputs[f"head_{sfx}"], np.int64)
        Ms.append(
            max(
                np.bincount(
                    h[(h >= RB * c) & (h < RB * (c + 1))] - RB * c, minlength=RB
                ).max()
                for c in range(NCORES)
            )
        )
    M = int(max(Ms))
    M += M & 1

    # per-core diag offsets (element index into [RB*E] fp16 out); pads add
    # 0.0 at element 1 (diag element index r*E + RB*c + r == 1 requires
    # RB*c == 1, never true)
    dgi_all, dgv_all = [], []
    for c in range(NCORES):
        dgi = np.full((128, NT), 1, np.int32)
        dgv = np.zeros((128, NT), np.float16)
        for ti in range(NT):
            rows = np.arange(ti * 128, min((ti + 1) * 128, RB), dtype=np.int64)
            p = rows - ti * 128
            dgi[p, ti] = (rows * E + RB * c + rows).astype(np.int32)
            dgv[p, ti] = 1.0
        dgi_all.append(dgi)
        dgv_all.append(dgv)

    in_maps = [dict() for _ in range(NCORES)]
    for s in ("a", "b"):
        sfx = "sr" if s == "a" else "tg"
        conf = np.asarray(inputs[f"conf_{sfx}"], np.float32)
        imp = np.asarray(inputs[f"imp_{sfx}"], np.float32)
        pca = np.asarray(inputs[f"pca_{sfx}"], np.float32)
        cip = np.empty((E, 3, E), np.float16)
        cip[:, 0, :] = conf
        cip[:, 1, :] = imp
        cip[:, 2, :] = pca
        for c in range(NCORES):
            m = in_maps[c]
            prep = _prep_set(
                ent[s],
                rel[s],
                inputs[f"relation_w_{sfx}"],
                inputs[f"head_{sfx}"],
                inputs[f"tail_{sfx}"],
                inputs[f"rel_{sfx}"],
                M,
                c,
            )
            m[f"cip_{s}"] = cip[RB * c : RB * (c + 1)]
            m[f"tab_{s}"] = prep["tab"]
            m[f"gix_{s}"] = prep["gix"]
            m[f"c0_{s}"] = prep["c0"]
            m[f"lidx_{s}"] = prep["lidx"]
            m["dgi"] = dgi_all[c]
            m["dgv"] = dgv_all[c]
    return M, in_maps


def kernel(**inputs):
    M, in_maps = _host_prep(inputs)
    runner = _get_runner(M)
    concat_in = runner.concat_inputs(in_maps)
    out_arrs = runner.run(concat_in)
    res = runner.split_outputs(out_arrs)
    adj_sr = np.concatenate(
        [res[c]["out_a"].astype(np.float32) for c in range(NCORES)], axis=0
    )
    adj_tg = np.concatenate(
        [res[c]["out_b"].astype(np.float32) for c in range(NCORES)], axis=0
    )
    return adj_sr, adj_tg
